# revision 1
# baseline (speedup 1.0000x reference)
"""Per-pixel adaptive 5x5 conv (KPN apply) on 8 Trainium2 NeuronCores.

out[b,c,h,w] = sum_{i,j} core[b,0,i*5+j,c,h,w] * frames[b,0,c,h+i-2,w+j-2]
(zero-padded borders), output [4,3,512,512] f32.

Sharding: pure data parallel, core k -> (b = k//2, H-half = k%2).

The metric is dominated by per-run host<->device transfer of the 315MB
`core` tensor, so inputs are compressed on the host before shipping:
  - core   -> int8, symmetric quantization with scale s = min(amax, 4*std)/127
             (the scale is folded into the frames, so the device kernel is
             just sum(q_t * f'_t));
  - frames -> bf16 (pre-scaled by s, zero-padded with halo rows host-side);
  - out    -> bf16 on device, upcast to f32 host-side.
Measured end-to-end rel err of this scheme vs the f32 reference: 1.051e-2
(gate 2e-2). int8 is the floor: int7 lands at ~1.9e-2 — no margin.

Device kernel (raw bass; this walrus build only allows one semaphore wait
per compute/DMA instruction so Tile auto-sync is unavailable): per
128-row block, one SWDGE cast-DMA (gpsimd) brings the 25 tap planes
[128, 25*512] straight from int8 DRAM into bf16 SBUF (contiguous 12.8KB
DRAM rows), and one HWDGE DMA (sync) brings a 5-row overlapping window
of the padded bf16 frame [128, 5, 516]. The DVE then runs entirely in
bf16 2x_1p mode (tensor_tensor's fastest uop on TRN2): ONE multiply over
a 4D access pattern covering all 25 taps, 6 contiguous tree-adds, and a
copy of the result row. Loads/stores are triple-buffered (NB=3; deeper
is timing-identical, NB=5 overflows SBUF); instructions on the same
engine execute in order, so intra-block chains need no semaphores.
Modeled exec ~99us/core against an ~84us DVE steady-state floor; the run
itself is dominated by shipping the 91MB of compressed inputs.
"""

import ml_dtypes
import numpy as np

import concourse.bass as bass
import concourse.mybir as mybir
from concourse.ap import AP
from concourse.bass_utils import run_bass_kernel_spmd

B, N, C, H, W = 4, 1, 3, 512, 512
K = 5
PAD = K // 2
NCORES = 8
HH = H // (NCORES // B)   # 256 rows per core
P = 128
NBLK_TOT = C * (HH // P)  # 6 blocks of 128 rows per core
WPAD = W + 2 * PAD        # 516
HHP = HH + 2 * PAD        # 260
TW = K * K * W            # 12800 elements per row: all 25 taps
NB = 3                    # pipeline buffer depth

BF16 = ml_dtypes.bfloat16

_CACHE = {}


def _build():
    nc = bass.Bass()
    bf16 = mybir.dt.bfloat16
    i8 = mybir.dt.int8

    fr = nc.declare_dram_parameter("fr", [C, HHP, WPAD], bf16, isOutput=False)
    co = nc.declare_dram_parameter("co", [C, HH, TW], i8, isOutput=False)
    out = nc.declare_dram_parameter("out", [C, HH, W], bf16, isOutput=True)

    def co_view(n):
        c, blk = n // (HH // P), n % (HH // P)
        return co[c, blk * P:blk * P + P, :]

    def fr_win(n):
        c, blk = n // (HH // P), n % (HH // P)
        fb = fr[c, blk * P:blk * P + P, :]
        return AP(fb.tensor, fb.offset, [(WPAD, P), (WPAD, K), (1, WPAD)])

    def out_view(n):
        c, blk = n // (HH // P), n % (HH // P)
        return out[c, blk * P:blk * P + P, :]

    from contextlib import ExitStack
    ctx = ExitStack()
    cbs = [ctx.enter_context(nc.sbuf_tensor(f"cb{i}", [P, K, K, W], bf16))
           for i in range(NB)]
    fts = [ctx.enter_context(nc.sbuf_tensor(f"ft{i}", [P, K, WPAD], bf16))
           for i in range(NB)]
    obs = [ctx.enter_context(nc.sbuf_tensor(f"ob{i}", [P, W], bf16))
           for i in range(NB)]
    with ctx:
        prod = ctx.enter_context(nc.sbuf_tensor("prod", [P, TW], bf16))
        dsem = ctx.enter_context(nc.semaphore("dsem"))  # ft loads (+16 per DMA)
        csem = ctx.enter_context(nc.semaphore("csem"))  # co cast loads (+16)
        osem = ctx.enter_context(nc.semaphore("osem"))  # stores (+16 per DMA)
        vsem = ctx.enter_context(nc.semaphore("vsem"))  # DVE block done (+1)
        block = ctx.enter_context(nc.Block())

        @block.sync
        def _(sync: bass.BassEngine):
            for n in range(NBLK_TOT):
                if n >= NB:
                    # DVE done with block n-NB => its buffers reusable,
                    # and ob[n-NB] ready to store.
                    sync.wait_ge(vsem, n - NB + 1)
                    sync.dma_start(
                        out=out_view(n - NB), in_=obs[n % NB][:]
                    ).then_inc(osem, 16)
                sync.dma_start(out=fts[n % NB][:], in_=fr_win(n)).then_inc(dsem, 16)
            for m in range(NBLK_TOT - NB, NBLK_TOT):
                sync.wait_ge(vsem, m + 1)
                sync.dma_start(out=out_view(m), in_=obs[m % NB][:]).then_inc(osem, 16)
            sync.wait_ge(osem, 16 * NBLK_TOT)

        @block.gpsimd
        def _(gpsimd: bass.BassEngine):
            for n in range(NBLK_TOT):
                if n >= NB:
                    gpsimd.wait_ge(vsem, n - NB + 1)   # mul(n-NB) freed cb
                # SWDGE cast-DMA: int8 DRAM -> bf16 SBUF
                gpsimd.dma_start(
                    out=cbs[n % NB][:], in_=co_view(n)
                ).then_inc(csem, 16)

        @block.vector
        def _(vector: bass.BassEngine):
            p0 = prod[:]
            pap = AP(p0.tensor, p0.offset,
                     [tuple(p0.ap[0]), (K * W, K), (W, K), (1, W)])
            for n in range(NBLK_TOT):
                cb, ft, ob = cbs[n % NB], fts[n % NB], obs[n % NB]
                vector.wait_ge(dsem, 16 * (n + 1))  # ft(n) loaded
                vector.wait_ge(csem, 16 * (n + 1))  # co(n) cast-loaded
                if n >= NB:
                    # store of block n-NB (same ob buffer) must be done
                    vector.wait_ge(osem, 16 * (n - NB + 1))
                f0 = ft[:, 0, 0:W]
                fap = AP(f0.tensor, f0.offset,
                         [tuple(f0.ap[0]), (WPAD, K), (1, K), (1, W)])
                # prod[p, i*5*W + j*W + w] = cb[p,i,j,w] * fr[p+i, j+w]
                vector.tensor_tensor(out=pap, in0=cb[:], in1=fap,
                                     op=mybir.AluOpType.mult)
                # tree-reduce the 25 tap planes (bf16 2x, contiguous slices)
                vector.tensor_add(out=prod[:, 0:12 * W],
                                  in0=prod[:, 0:12 * W],
                                  in1=prod[:, 12 * W:24 * W])
                vector.tensor_add(out=prod[:, 0:6 * W],
                                  in0=prod[:, 0:6 * W],
                                  in1=prod[:, 6 * W:12 * W])
                vector.tensor_add(out=prod[:, 0:3 * W],
                                  in0=prod[:, 0:3 * W],
                                  in1=prod[:, 3 * W:6 * W])
                vector.tensor_add(out=prod[:, 0:W],
                                  in0=prod[:, 0:W], in1=prod[:, W:2 * W])
                vector.tensor_add(out=prod[:, 0:W],
                                  in0=prod[:, 0:W], in1=prod[:, 2 * W:3 * W])
                vector.tensor_add(out=prod[:, 0:W],
                                  in0=prod[:, 0:W], in1=prod[:, 24 * W:25 * W])
                vector.tensor_copy(out=ob[:], in_=prod[:, 0:W]).then_inc(vsem, 1)
    return nc


def get_nc():
    if "nc" not in _CACHE:
        _CACHE["nc"] = _build()
    return _CACHE["nc"]


def shard_inputs(frames, core):
    frames = np.asarray(frames, dtype=np.float32)
    core = np.asarray(core, dtype=np.float32)
    # sampled std is plenty for picking the clip threshold (~3M samples)
    sd = float(core.ravel()[::101].std())
    amax = float(max(core.max(), -core.min()))
    s = min(amax, 4.0 * sd) / 127.0 if amax > 0 else 1.0
    inv = np.float32(1.0 / s)
    frs = (frames * np.float32(s)).astype(BF16)  # scale folded into frames
    in_maps = []
    for k in range(NCORES):
        b, half = k // 2, k % 2
        h0 = half * HH
        frp = np.zeros((C, HHP, WPAD), BF16)
        lo, hi = h0 - PAD, h0 + HH + PAD
        clo, chi = max(lo, 0), min(hi, H)
        frp[:, clo - lo:clo - lo + chi - clo, PAD:PAD + W] = frs[b, 0, :, clo:chi, :]
        # [25, C, HH, W] -> [C, HH, 25, W], quantize to int8
        sl = core[b, 0, :, :, h0:h0 + HH, :].transpose(1, 2, 0, 3)
        qt = sl * inv
        np.rint(qt, out=qt)
        np.clip(qt, -127, 127, out=qt)
        coq = np.ascontiguousarray(qt.astype(np.int8)).reshape(C, HH, TW)
        in_maps.append({"fr": frp, "co": coq})
    return in_maps


def run(in_maps, **kwargs):
    return run_bass_kernel_spmd(get_nc(), in_maps, list(range(NCORES)), **kwargs)


def kernel(frames, core):
    in_maps = shard_inputs(frames, core)
    # Transient device wedges (NRT_EXEC_UNIT_UNRECOVERABLE etc.) clear on
    # re-run; retry rather than failing the whole call on infra noise.
    last_err = None
    for attempt in range(3):
        try:
            res = run(in_maps).results
            break
        except Exception as e:  # noqa: BLE001 - any runtime failure retries
            last_err = e
    else:
        raise last_err
    outp = np.empty((B, C, H, W), np.float32)
    for k in range(NCORES):
        b, half = k // 2, k % 2
        outp[b, :, half * HH:(half + 1) * HH, :] = res[k]["out"]
    return outp



# revision 3
# speedup vs baseline: 34.5297x; 34.5297x over previous
"""Per-pixel adaptive 5x5 conv (KPN apply) on 8 Trainium2 NeuronCores.

out[b,c,h,w] = sum_{i,j} core[b,0,i*5+j,c,h,w] * frames[b,0,c,h+i-2,w+j-2]
(zero-padded borders), output [4,3,512,512] f32.

Sharding: pure data parallel, core k -> (b = k//2, H-half = k%2).

The metric is dominated by per-run host<->device transfer of the 315MB
`core` tensor, so inputs are compressed on the host before shipping:
  - core   -> int8, symmetric quantization with scale s = min(amax, 4*std)/127
             (the scale is folded into the frames, so the device kernel is
             just sum(q_t * f'_t));
  - frames -> bf16 (pre-scaled by s, zero-padded with halo rows host-side);
  - out    -> bf16 on device, upcast to f32 host-side.
Measured end-to-end rel err of this scheme vs the f32 reference: 1.051e-2
(gate 2e-2). int8 is the floor: int7 lands at ~1.9e-2 — no margin.

Device kernel (raw bass; this walrus build only allows one semaphore wait
per compute/DMA instruction so Tile auto-sync is unavailable): per
128-row block, one SWDGE cast-DMA (gpsimd) brings the 25 tap planes
[128, 25*512] straight from int8 DRAM into bf16 SBUF (contiguous 12.8KB
DRAM rows), and one HWDGE DMA (sync) brings a 5-row overlapping window
of the padded bf16 frame [128, 5, 516]. The DVE then runs entirely in
bf16 2x_1p mode (tensor_tensor's fastest uop on TRN2): ONE multiply over
a 4D access pattern covering all 25 taps, 6 contiguous tree-adds, and a
copy of the result row. Loads/stores are triple-buffered (NB=3; deeper
is timing-identical, NB=5 overflows SBUF); instructions on the same
engine execute in order, so intra-block chains need no semaphores.
Modeled exec ~99us/core against an ~84us DVE steady-state floor; the run
itself is dominated by shipping the 91MB of compressed inputs.
"""

import ml_dtypes
import numpy as np

import concourse.bass as bass
import concourse.mybir as mybir
from concourse.ap import AP
from concourse.bass_utils import run_bass_kernel_spmd

B, N, C, H, W = 4, 1, 3, 512, 512
K = 5
PAD = K // 2
NCORES = 8
HH = H // (NCORES // B)   # 256 rows per core
P = 128
NBLK_TOT = C * (HH // P)  # 6 blocks of 128 rows per core
WPAD = W + 2 * PAD        # 516
HHP = HH + 2 * PAD        # 260
TW = K * K * W            # 12800 elements per row: all 25 taps
NB = 3                    # pipeline buffer depth

BF16 = ml_dtypes.bfloat16

_CACHE = {}


def _build(reps=1):
    # `reps` repeats the whole program back-to-back inside one NEFF
    # (same inputs, same outputs rewritten) — bench-only, used to
    # measure marginal per-exec device time with dispatch differenced
    # out. The graded path always uses reps=1.
    nc = bass.Bass()
    bf16 = mybir.dt.bfloat16
    i8 = mybir.dt.int8
    NT = NBLK_TOT * reps

    fr = nc.declare_dram_parameter("fr", [C, HHP, WPAD], bf16, isOutput=False)
    co = nc.declare_dram_parameter("co", [C, HH, TW], i8, isOutput=False)
    out = nc.declare_dram_parameter("out", [C, HH, W], bf16, isOutput=True)

    def co_view(n):
        n %= NBLK_TOT
        c, blk = n // (HH // P), n % (HH // P)
        return co[c, blk * P:blk * P + P, :]

    def fr_win(n):
        n %= NBLK_TOT
        c, blk = n // (HH // P), n % (HH // P)
        fb = fr[c, blk * P:blk * P + P, :]
        return AP(fb.tensor, fb.offset, [(WPAD, P), (WPAD, K), (1, WPAD)])

    def out_view(n):
        n %= NBLK_TOT
        c, blk = n // (HH // P), n % (HH // P)
        return out[c, blk * P:blk * P + P, :]

    from contextlib import ExitStack
    ctx = ExitStack()
    cbs = [ctx.enter_context(nc.sbuf_tensor(f"cb{i}", [P, K, K, W], bf16))
           for i in range(NB)]
    fts = [ctx.enter_context(nc.sbuf_tensor(f"ft{i}", [P, K, WPAD], bf16))
           for i in range(NB)]
    obs = [ctx.enter_context(nc.sbuf_tensor(f"ob{i}", [P, W], bf16))
           for i in range(NB)]
    with ctx:
        prod = ctx.enter_context(nc.sbuf_tensor("prod", [P, TW], bf16))
        dsem = ctx.enter_context(nc.semaphore("dsem"))  # ft loads (+16 per DMA)
        csem = ctx.enter_context(nc.semaphore("csem"))  # co cast loads (+16)
        osem = ctx.enter_context(nc.semaphore("osem"))  # stores (+16 per DMA)
        vsem = ctx.enter_context(nc.semaphore("vsem"))  # DVE block done (+1)
        block = ctx.enter_context(nc.Block())

        @block.sync
        def _(sync: bass.BassEngine):
            for n in range(NT):
                if n >= NB:
                    # DVE done with block n-NB => its buffers reusable,
                    # and ob[n-NB] ready to store.
                    sync.wait_ge(vsem, n - NB + 1)
                    sync.dma_start(
                        out=out_view(n - NB), in_=obs[(n - NB) % NB][:]
                    ).then_inc(osem, 16)
                sync.dma_start(out=fts[n % NB][:], in_=fr_win(n)).then_inc(dsem, 16)
            for m in range(NT - NB, NT):
                sync.wait_ge(vsem, m + 1)
                sync.dma_start(out=out_view(m), in_=obs[m % NB][:]).then_inc(osem, 16)
            sync.wait_ge(osem, 16 * NT)

        @block.gpsimd
        def _(gpsimd: bass.BassEngine):
            for n in range(NT):
                if n >= NB:
                    gpsimd.wait_ge(vsem, n - NB + 1)   # mul(n-NB) freed cb
                # SWDGE cast-DMA: int8 DRAM -> bf16 SBUF
                gpsimd.dma_start(
                    out=cbs[n % NB][:], in_=co_view(n)
                ).then_inc(csem, 16)

        @block.vector
        def _(vector: bass.BassEngine):
            p0 = prod[:]
            pap = AP(p0.tensor, p0.offset,
                     [tuple(p0.ap[0]), (K * W, K), (W, K), (1, W)])
            for n in range(NT):
                cb, ft, ob = cbs[n % NB], fts[n % NB], obs[n % NB]
                vector.wait_ge(dsem, 16 * (n + 1))  # ft(n) loaded
                vector.wait_ge(csem, 16 * (n + 1))  # co(n) cast-loaded
                if n >= NB:
                    # store of block n-NB (same ob buffer) must be done
                    vector.wait_ge(osem, 16 * (n - NB + 1))
                f0 = ft[:, 0, 0:W]
                fap = AP(f0.tensor, f0.offset,
                         [tuple(f0.ap[0]), (WPAD, K), (1, K), (1, W)])
                # prod[p, i*5*W + j*W + w] = cb[p,i,j,w] * fr[p+i, j+w]
                vector.tensor_tensor(out=pap, in0=cb[:], in1=fap,
                                     op=mybir.AluOpType.mult)
                # tree-reduce the 25 tap planes (bf16 2x, contiguous slices)
                vector.tensor_add(out=prod[:, 0:12 * W],
                                  in0=prod[:, 0:12 * W],
                                  in1=prod[:, 12 * W:24 * W])
                vector.tensor_add(out=prod[:, 0:6 * W],
                                  in0=prod[:, 0:6 * W],
                                  in1=prod[:, 6 * W:12 * W])
                vector.tensor_add(out=prod[:, 0:3 * W],
                                  in0=prod[:, 0:3 * W],
                                  in1=prod[:, 3 * W:6 * W])
                vector.tensor_add(out=prod[:, 0:W],
                                  in0=prod[:, 0:W], in1=prod[:, W:2 * W])
                vector.tensor_add(out=prod[:, 0:W],
                                  in0=prod[:, 0:W], in1=prod[:, 2 * W:3 * W])
                vector.tensor_add(out=prod[:, 0:W],
                                  in0=prod[:, 0:W], in1=prod[:, 24 * W:25 * W])
                vector.tensor_copy(out=ob[:], in_=prod[:, 0:W]).then_inc(vsem, 1)
    return nc


def get_nc(reps=1):
    key = f"nc{reps}"
    if key not in _CACHE:
        _CACHE[key] = _build(reps)
    return _CACHE[key]


def shard_inputs(frames, core):
    frames = np.asarray(frames, dtype=np.float32)
    core = np.asarray(core, dtype=np.float32)
    # sampled std is plenty for picking the clip threshold (~3M samples)
    sd = float(core.ravel()[::101].std())
    amax = float(max(core.max(), -core.min()))
    s = min(amax, 4.0 * sd) / 127.0 if amax > 0 else 1.0
    inv = np.float32(1.0 / s)
    frs = (frames * np.float32(s)).astype(BF16)  # scale folded into frames
    in_maps = []
    for k in range(NCORES):
        b, half = k // 2, k % 2
        h0 = half * HH
        frp = np.zeros((C, HHP, WPAD), BF16)
        lo, hi = h0 - PAD, h0 + HH + PAD
        clo, chi = max(lo, 0), min(hi, H)
        frp[:, clo - lo:clo - lo + chi - clo, PAD:PAD + W] = frs[b, 0, :, clo:chi, :]
        # [25, C, HH, W] -> [C, HH, 25, W], quantize to int8
        sl = core[b, 0, :, :, h0:h0 + HH, :].transpose(1, 2, 0, 3)
        qt = sl * inv
        np.rint(qt, out=qt)
        np.clip(qt, -127, 127, out=qt)
        coq = np.ascontiguousarray(qt.astype(np.int8)).reshape(C, HH, TW)
        in_maps.append({"fr": frp, "co": coq})
    return in_maps


def run(in_maps, **kwargs):
    return run_bass_kernel_spmd(get_nc(), in_maps, list(range(NCORES)), **kwargs)


def kernel(frames, core):
    in_maps = shard_inputs(frames, core)
    # Transient device wedges (NRT_EXEC_UNIT_UNRECOVERABLE etc.) clear on
    # re-run; retry rather than failing the whole call on infra noise.
    last_err = None
    for attempt in range(3):
        try:
            res = run(in_maps).results
            break
        except Exception as e:  # noqa: BLE001 - any runtime failure retries
            last_err = e
    else:
        raise last_err
    outp = np.empty((B, C, H, W), np.float32)
    for k in range(NCORES):
        b, half = k // 2, k % 2
        outp[b, :, half * HH:(half + 1) * HH, :] = res[k]["out"]
    return outp

